# revision 35
# baseline (speedup 1.0000x reference)
"""Trainium2 Bass kernel for nn_Centroid (segment_reduce + EMA).

Computes, for full inputs:
    sums   = segment_sum(embed, y, C)            # [C, D]
    counts = segment_sum(ones,  y, C)            # [C]
    out    = THETA*centroid + (1-THETA) * sums/(counts+EPS)

Sharding strategy (class-sharded; host does the shard gather):
  Core i owns classes [i*125, (i+1)*125). The host routes each batch row to
  the core owning its class.

Device-side pipeline:
  - Embed rows stream in fp8-e4m3 packed TOGETHER with their host-built raw
    one-hot rows: each 128-row k-tile is [128, 1152] = [embed 1024 | onehot
    128].  Chunks ([2,4,4,...,2,2] tiles) alternate the two HWDGE queues
    (sync/scalar); the odd-sized lead staggers the queues so completions
    arrive in consumption order at ~430 GB/s aggregate.
  - PE consumes tiles in PAIRS with fp8 DoubleRow matmuls (both operands
    e4m3) - 2 k-tiles per matmul at 2x rate.  PSUM accumulates raw per-class
    sums.
  - The EMA term is folded in as a final matmul pair:
    psum += diag(theta*(count+eps)/(1-theta)) @ cent   (fp16)
    so that the single epilogue scale psum * w  (w = (1-theta)/(count+eps))
    yields  (1-theta)*sums/count + theta*cent  directly.
  - Epilogue: ps0 scaled on DVE, ps1 on ACT (parallel), two output DMAs on
    separate queues; ps0's chain is ordered first so its output DMA overlaps
    ps1's final matmuls.
  - Full-width warm-up matmuls keep the PE busy from program start until
    ~3 chunks have landed, so the HAM p-state reaches 2.4 GHz and the
    stream then runs gap-free at 216 ns per DoubleRow matmul.
  - The unused Pool SWDGE queue is shrunk to 1 ring: the toolchain's fixed
    end-of-program teardown includes per-ring waits.
"""

import os

import ml_dtypes
import numpy as np

import concourse.bacc as bacc
import concourse.mybir as mybir
import concourse.tile as tile
from concourse.bass_utils import run_bass_kernel_spmd

NCORES = 8
B = 16384
C = 1000
D = 1024
CPC = C // NCORES  # classes per core = 125
P = 128
W = 1152  # packed tile width: 1024 embed + 128 onehot
THETA = 0.7
EPS = 1e-8
NWARM = 13  # bridging PE warm-up matmuls (512-col)
NPOST = 18  # post-kernel PE filler matmuls (keep clocks up through teardown)

_NC_CACHE: dict[int, object] = {}

# test.py sets KERNEL_TRACE=1 to collect an NTFF profile; results stashed here.
LAST_RESULTS = None


def _build_nc(T: int):
    """Build + compile the per-core Bass program for T (even) 128-row tiles."""
    f32 = mybir.dt.float32
    f16 = mybir.dt.float16
    f8 = mybir.dt.float8e4
    # chunk plan in TILES: one 2-tile lead, then 4-tile chunks, remainder
    # as 2-tile tail chunks. With alternating queues this staggers the two
    # queues' completion times so chunks arrive in consumption order.
    plan = [2]
    rem = T - 2
    while rem >= 4:
        plan.append(4)
        rem -= 4
    while rem > 0:
        c = min(2, rem)
        plan.append(c)
        rem -= c
    while len(plan) >= 3 and plan[-1] == 4:
        plan[-1] = 2
        plan.append(2)

    nc = bacc.Bacc(
        "TRN2",
        target_bir_lowering=False,
        debug=False,
        enable_asserts=False,
        num_devices=NCORES,
    )
    # Shrink DMA ring counts: the NEFF epilogue's per-ring drain/semaphore
    # waits dominate the teardown tail. Pool SWDGE is unused (1 ring); the
    # two HWDGE queues keep 12 rings each (enough for ~300 GB/s per queue).
    nrings = {"qPoolDynamic": 1}
    nc.m.queues = [
        mybir.DMAQueue(
            type=q.type,
            name=q.name,
            blocks=[],
            engine=q.engine,
            location_alt=q.location_alt,
            num_queues=nrings.get(q.name, q.num_queues),
            is_HWDGE=q.is_HWDGE,
            num_semaphores=0,
            semaphores=[],
        )
        for q in nc.m.queues
    ]
    # packed embed+onehot chunks, partition-major inside each chunk
    emb_ds = [
        nc.dram_tensor(f"emb{j}", [P, ntj * W], f8, kind="ExternalInput")
        for j, ntj in enumerate(plan)
    ]
    # emc: [0:128] thi = diag(theta*(cnt+eps)/(1-theta)), [128:1152] cent,
    #      [1152] per-class w column
    emc_d = nc.dram_tensor("emc", [P, W + 128], f16, kind="ExternalInput")
    out_d = nc.dram_tensor("out", [P, D], f16, kind="ExternalOutput")

    with tile.TileContext(nc) as tc:
        # --- embed stream: chunk buffers are RAW SBUF tensors allocated
        # before any tile pool opens, so the dma_starts are the first
        # instructions on the sync/scalar queues - ahead of the pool-entry
        # barrier. Chunks alternate queues; emc rides last on scalar.
        queue_items = {0: [], 1: []}  # 0=sync, 1=scalar
        for j in range(len(plan)):
            queue_items[j % 2].append(("chunk", j))
        queue_items[1].append(("emc", None))

        emc_t = nc.alloc_sbuf_tensor("emcbuf", [P, W + 128], f16)
        gbufs = {
            j: nc.alloc_sbuf_tensor(f"gbuf{j}", [P, ntj, W], f8)
            for j, ntj in enumerate(plan)
        }
        for qi, eng in ((0, nc.sync), (1, nc.scalar)):
            for kind, j in queue_items[qi]:
                if kind == "chunk":
                    eng.dma_start(out=gbufs[j][:], in_=emb_ds[j][:])
                else:
                    eng.dma_start(out=emc_t[:], in_=emc_d[:])
        tiles_by_chunk = gbufs

        with (
            tc.tile_pool(name="const", bufs=1) as cpool,
            tc.tile_pool(name="psum", bufs=1, space="PSUM") as psum,
        ):
            # --- bridging PE warm-up: full-width matmuls keep the PE busy
            # from program start until ~3 chunks have landed; the HAM p-state
            # ramps to 2.4 GHz and the stream then runs gap-free at full rate
            wa = cpool.tile([P, 512], f16)
            nc.vector.memset(wa[:], 1.0)
            scratch = psum.tile([P, 512], f32)
            for _ in range(NWARM):
                nc.tensor.matmul(
                    scratch[0:64, :], lhsT=wa[:, 0:64], rhs=wa[:],
                    start=True, stop=True,
                )

            ps0 = psum.tile([P, 512], f32)
            ps1 = psum.tile([P, 512], f32)

            dr = mybir.MatmulPerfMode.DoubleRow
            units = []  # (pair_ap | single_tile_ap, is_pair)
            for j in range(len(plan)):
                g = tiles_by_chunk[j]
                nt = plan[j]
                for q in range(nt // 2):
                    units.append((g[:, 2 * q : 2 * q + 2, :], True))
                if nt % 2:
                    units.append((g[:, nt - 1, :], False))

            def mm(ps, u, lo, hi, st):
                pg, is_pair = u
                if is_pair:
                    nc.tensor.matmul(
                        ps[:], lhsT=pg[:, :, 1024:1152], rhs=pg[:, :, lo:hi],
                        start=st, stop=False, perf_mode=dr,
                    )
                else:
                    nc.tensor.matmul(
                        ps[:], lhsT=pg[:, 1024:1152], rhs=pg[:, lo:hi],
                        start=st, stop=False,
                    )

            for k, u in enumerate(units[:-1]):
                mm(ps0, u, 0, 512, k == 0)
                mm(ps1, u, 512, 1024, k == 0)
            last = units[-1]
            one = len(units) == 1
            # finish ps0 first so its epilogue + output DMA overlap the
            # remaining ps1 matmuls
            mm(ps0, last, 0, 512, one)
            nc.tensor.matmul(
                ps0[:], lhsT=emc_t[:, 0:128], rhs=emc_t[:, 128:640],
                start=False, stop=True,
            )
            res = cpool.tile([P, D], f16)
            # w stored as fp32 bit-pattern in two fp16 columns
            wcol = emc_t[:, 1152:1154].bitcast(f32)
            nc.vector.tensor_scalar(
                out=res[:, 0:512], in0=ps0[:], scalar1=wcol, scalar2=None,
                op0=mybir.AluOpType.mult,
            )
            nc.sync.dma_start(out=out_d[:, 0:512], in_=res[:, 0:512])

            mm(ps1, last, 512, 1024, one)
            nc.tensor.matmul(
                ps1[:], lhsT=emc_t[:, 0:128], rhs=emc_t[:, 640:1152],
                start=False, stop=True,
            )
            nc.scalar.mul(res[:, 512:1024], ps1[:], wcol)
            nc.scalar.dma_start(out=out_d[:, 512:1024], in_=res[:, 512:1024])

            # --- post-kernel PE filler: keep the HAM p-state at full clock
            # while the output DMA drains, so the toolchain's fixed ~250
            # teardown instructions execute at full rate instead of the
            # quarter-clock state HAM drops to ~3.4us after the PE idles.
            # rhs reads `res` so these schedule strictly AFTER the epilogue
            # (dep-free fillers get hoisted between real matmuls and clobber
            # their loaded weights).
            for _ in range(NPOST):
                nc.tensor.matmul(
                    scratch[0:64, :], lhsT=wa[:, 0:64], rhs=res[:, 512:1024],
                    start=True, stop=True,
                )

    nc.compile()
    return nc


def _shard_inputs(embed: np.ndarray, y: np.ndarray, centroid: np.ndarray):
    """Host-side sharding: route each batch row to its class-owner core."""
    f8 = ml_dtypes.float8_e4m3
    y64 = np.asarray(y).astype(np.int64).ravel()
    owner = y64 // CPC
    order = np.argsort(owner, kind="stable")
    core_counts = np.bincount(owner, minlength=NCORES)
    cls_counts = np.bincount(y64, minlength=C).astype(np.float64)
    T = max(int(-(-core_counts.max() // P)), 2)
    T += T % 2  # even tiles: DoubleRow pairs only
    n_pad = T * P
    plan = [2]
    rem = T - 2
    while rem >= 4:
        plan.append(4)
        rem -= 4
    while rem > 0:
        c = min(2, rem)
        plan.append(c)
        rem -= c
    while len(plan) >= 3 and plan[-1] == 4:
        plan[-1] = 2
        plan.append(2)

    # per-class EMA scales
    w_all = (1.0 - THETA) / (cls_counts + EPS)  # [C]
    thi_diag = THETA * (cls_counts + EPS) / (1.0 - THETA)  # [C]

    eye8 = np.zeros((P + 1, P), dtype=f8)
    eye8[np.arange(P), np.arange(P)] = 1.0

    in_maps = []
    start = 0
    for i in range(NCORES):
        n_i = int(core_counts[i])
        rows_i = order[start : start + n_i]
        start += n_i
        cls_i = y64[rows_i] - i * CPC  # local class in [0,125)

        # packed [T*128, 1152]: embed fp8 | raw one-hot fp8
        packed = np.zeros((n_pad, W), dtype=f8)
        packed[:n_i, 0:1024] = embed[rows_i].astype(f8)
        packed[:n_i, 1024:1152] = eye8[cls_i]
        # partition-major per tile: [128, T, 1152]
        pm = packed.reshape(T, P, W).transpose(1, 0, 2)

        m = {}
        t0 = 0
        for j, ntj in enumerate(plan):
            m[f"emb{j}"] = np.ascontiguousarray(
                pm[:, t0 : t0 + ntj, :].reshape(P, ntj * W)
            )
            t0 += ntj

        emc = np.zeros((P, W + 128), dtype=np.float16)
        dg = thi_diag[i * CPC : (i + 1) * CPC].astype(np.float16)
        emc[np.arange(CPC), np.arange(CPC)] = dg
        emc[:CPC, 128:1152] = centroid[i * CPC : (i + 1) * CPC].astype(np.float16)
        w32 = np.zeros((P, 1), dtype=np.float32)
        w32[:CPC, 0] = w_all[i * CPC : (i + 1) * CPC].astype(np.float32)
        emc[:, 1152:1154] = w32.view(np.float16)
        m["emc"] = emc
        in_maps.append(m)
    return in_maps, T, cls_counts


def kernel(embed: np.ndarray, y: np.ndarray, centroid: np.ndarray) -> np.ndarray:
    global LAST_RESULTS
    embed = np.ascontiguousarray(np.asarray(embed, dtype=np.float32))
    centroid = np.ascontiguousarray(np.asarray(centroid, dtype=np.float32))

    in_maps, T, cls_counts = _shard_inputs(embed, y, centroid)
    if T not in _NC_CACHE:
        _NC_CACHE[T] = _build_nc(T)
    nc = _NC_CACHE[T]

    trace = os.environ.get("KERNEL_TRACE", "0") == "1"
    res = run_bass_kernel_spmd(
        nc, in_maps, core_ids=list(range(NCORES)), trace=trace
    )
    LAST_RESULTS = res
    out = np.concatenate(
        [res.results[i]["out"][:CPC] for i in range(NCORES)], axis=0
    ).astype(np.float32)
    # empty classes: the fp16 diag underflows; patch exactly on host
    empty = np.where(cls_counts == 0)[0]
    if empty.size:
        out[empty] = THETA * centroid[empty]
    return out


# revision 36
# speedup vs baseline: 1.1369x; 1.1369x over previous
"""Trainium2 Bass kernel for nn_Centroid (segment_reduce + EMA).

Computes, for full inputs:
    sums   = segment_sum(embed, y, C)            # [C, D]
    counts = segment_sum(ones,  y, C)            # [C]
    out    = THETA*centroid + (1-THETA) * sums/(counts+EPS)

Sharding strategy (class-sharded; host does the shard gather):
  Core i owns classes [i*125, (i+1)*125). The host routes each batch row to
  the core owning its class.

Device-side pipeline:
  - Embed rows stream in fp8-e4m3 packed TOGETHER with their host-built raw
    one-hot rows: each 128-row k-tile is [128, 1152] = [embed 1024 | onehot
    128].  Chunks ([2,4,4,...,2,2] tiles) alternate the two HWDGE queues
    (sync/scalar); the odd-sized lead staggers the queues so completions
    arrive in consumption order at ~430 GB/s aggregate.
  - PE consumes tiles in PAIRS with fp8 DoubleRow matmuls (both operands
    e4m3) - 2 k-tiles per matmul at 2x rate.  PSUM accumulates raw per-class
    sums.
  - The EMA term is folded in as a final matmul pair:
    psum += diag(theta*(count+eps)/(1-theta)) @ cent   (fp16)
    so that the single epilogue scale psum * w  (w = (1-theta)/(count+eps))
    yields  (1-theta)*sums/count + theta*cent  directly.
  - Epilogue: ps0 scaled on DVE, ps1 on ACT (parallel), two output DMAs on
    separate queues; ps0's chain is ordered first so its output DMA overlaps
    ps1's final matmuls.
  - Full-width warm-up matmuls keep the PE busy from program start until
    ~3 chunks have landed, so the HAM p-state reaches 2.4 GHz and the
    stream then runs gap-free at 216 ns per DoubleRow matmul.
  - The unused Pool SWDGE queue is shrunk to 1 ring: the toolchain's fixed
    end-of-program teardown includes per-ring waits.
"""

import os

import ml_dtypes
import numpy as np

import concourse.bacc as bacc
import concourse.mybir as mybir
import concourse.tile as tile
from concourse.bass_utils import run_bass_kernel_spmd

NCORES = 8
B = 16384
C = 1000
D = 1024
CPC = C // NCORES  # classes per core = 125
P = 128
W = 1152  # packed tile width: 1024 embed + 128 onehot
THETA = 0.7
EPS = 1e-8
NWARM = 13  # bridging PE warm-up matmuls (512-col)

_NC_CACHE: dict[int, object] = {}

# test.py sets KERNEL_TRACE=1 to collect an NTFF profile; results stashed here.
LAST_RESULTS = None


def _build_nc(T: int):
    """Build + compile the per-core Bass program for T (even) 128-row tiles."""
    f32 = mybir.dt.float32
    f16 = mybir.dt.float16
    f8 = mybir.dt.float8e4
    # chunk plan in TILES: one 2-tile lead, then 4-tile chunks, remainder
    # as 2-tile tail chunks. With alternating queues this staggers the two
    # queues' completion times so chunks arrive in consumption order.
    plan = [2]
    rem = T - 2
    while rem >= 4:
        plan.append(4)
        rem -= 4
    while rem > 0:
        c = min(2, rem)
        plan.append(c)
        rem -= c
    while len(plan) >= 3 and plan[-1] == 4:
        plan[-1] = 2
        plan.append(2)

    nc = bacc.Bacc(
        "TRN2",
        target_bir_lowering=False,
        debug=False,
        enable_asserts=False,
        num_devices=NCORES,
    )
    # Shrink DMA ring counts: the NEFF epilogue's per-ring drain/semaphore
    # waits dominate the teardown tail. Pool SWDGE is unused (1 ring); the
    # two HWDGE queues keep 12 rings each (enough for ~300 GB/s per queue).
    nrings = {"qPoolDynamic": 1}
    nc.m.queues = [
        mybir.DMAQueue(
            type=q.type,
            name=q.name,
            blocks=[],
            engine=q.engine,
            location_alt=q.location_alt,
            num_queues=nrings.get(q.name, q.num_queues),
            is_HWDGE=q.is_HWDGE,
            num_semaphores=0,
            semaphores=[],
        )
        for q in nc.m.queues
    ]
    # packed embed+onehot chunks, partition-major inside each chunk
    emb_ds = [
        nc.dram_tensor(f"emb{j}", [P, ntj * W], f8, kind="ExternalInput")
        for j, ntj in enumerate(plan)
    ]
    # emc: [0:128] thi = diag(theta*(cnt+eps)/(1-theta)), [128:1152] cent,
    #      [1152] per-class w column
    emc_d = nc.dram_tensor("emc", [P, W + 128], f16, kind="ExternalInput")
    out_d = nc.dram_tensor("out", [P, D], f16, kind="ExternalOutput")

    with tile.TileContext(nc) as tc:
        # --- embed stream: chunk buffers are RAW SBUF tensors allocated
        # before any tile pool opens, so the dma_starts are the first
        # instructions on the sync/scalar queues - ahead of the pool-entry
        # barrier. Chunks alternate queues; emc rides last on scalar.
        queue_items = {0: [], 1: []}  # 0=sync, 1=scalar
        for j in range(len(plan)):
            queue_items[j % 2].append(("chunk", j))
        queue_items[1].append(("emc", None))

        emc_t = nc.alloc_sbuf_tensor("emcbuf", [P, W + 128], f16)
        gbufs = {
            j: nc.alloc_sbuf_tensor(f"gbuf{j}", [P, ntj, W], f8)
            for j, ntj in enumerate(plan)
        }
        for qi, eng in ((0, nc.sync), (1, nc.scalar)):
            for kind, j in queue_items[qi]:
                if kind == "chunk":
                    eng.dma_start(out=gbufs[j][:], in_=emb_ds[j][:])
                else:
                    eng.dma_start(out=emc_t[:], in_=emc_d[:])
        tiles_by_chunk = gbufs

        with (
            tc.tile_pool(name="const", bufs=1) as cpool,
            tc.tile_pool(name="psum", bufs=1, space="PSUM") as psum,
        ):
            # --- bridging PE warm-up: full-width matmuls keep the PE busy
            # from program start until ~3 chunks have landed; the HAM p-state
            # ramps to 2.4 GHz and the stream then runs gap-free at full rate
            wa = cpool.tile([P, 512], f16)
            nc.vector.memset(wa[:], 1.0)
            scratch = psum.tile([P, 512], f32)
            for _ in range(NWARM):
                nc.tensor.matmul(
                    scratch[0:64, :], lhsT=wa[:, 0:64], rhs=wa[:],
                    start=True, stop=True,
                )

            ps0 = psum.tile([P, 512], f32)
            ps1 = psum.tile([P, 512], f32)

            dr = mybir.MatmulPerfMode.DoubleRow
            units = []  # (pair_ap | single_tile_ap, is_pair)
            for j in range(len(plan)):
                g = tiles_by_chunk[j]
                nt = plan[j]
                for q in range(nt // 2):
                    units.append((g[:, 2 * q : 2 * q + 2, :], True))
                if nt % 2:
                    units.append((g[:, nt - 1, :], False))

            def mm(ps, u, lo, hi, st):
                pg, is_pair = u
                if is_pair:
                    nc.tensor.matmul(
                        ps[:], lhsT=pg[:, :, 1024:1152], rhs=pg[:, :, lo:hi],
                        start=st, stop=False, perf_mode=dr,
                    )
                else:
                    nc.tensor.matmul(
                        ps[:], lhsT=pg[:, 1024:1152], rhs=pg[:, lo:hi],
                        start=st, stop=False,
                    )

            for k, u in enumerate(units[:-1]):
                mm(ps0, u, 0, 512, k == 0)
                mm(ps1, u, 512, 1024, k == 0)
            last = units[-1]
            one = len(units) == 1
            # finish ps0 first so its epilogue + output DMA overlap the
            # remaining ps1 matmuls
            mm(ps0, last, 0, 512, one)
            nc.tensor.matmul(
                ps0[:], lhsT=emc_t[:, 0:128], rhs=emc_t[:, 128:640],
                start=False, stop=True,
            )
            res = cpool.tile([P, D], f16)
            # w stored as fp32 bit-pattern in two fp16 columns
            wcol = emc_t[:, 1152:1154].bitcast(f32)
            nc.vector.tensor_scalar(
                out=res[:, 0:512], in0=ps0[:], scalar1=wcol, scalar2=None,
                op0=mybir.AluOpType.mult,
            )
            nc.sync.dma_start(out=out_d[:, 0:512], in_=res[:, 0:512])

            mm(ps1, last, 512, 1024, one)
            nc.tensor.matmul(
                ps1[:], lhsT=emc_t[:, 0:128], rhs=emc_t[:, 640:1152],
                start=False, stop=True,
            )
            nc.scalar.mul(res[:, 512:1024], ps1[:], wcol)
            nc.scalar.dma_start(out=out_d[:, 512:1024], in_=res[:, 512:1024])


    nc.compile()
    return nc


def _shard_inputs(embed: np.ndarray, y: np.ndarray, centroid: np.ndarray):
    """Host-side sharding: route each batch row to its class-owner core."""
    f8 = ml_dtypes.float8_e4m3
    y64 = np.asarray(y).astype(np.int64).ravel()
    owner = y64 // CPC
    order = np.argsort(owner, kind="stable")
    core_counts = np.bincount(owner, minlength=NCORES)
    cls_counts = np.bincount(y64, minlength=C).astype(np.float64)
    T = max(int(-(-core_counts.max() // P)), 2)
    T += T % 2  # even tiles: DoubleRow pairs only
    n_pad = T * P
    plan = [2]
    rem = T - 2
    while rem >= 4:
        plan.append(4)
        rem -= 4
    while rem > 0:
        c = min(2, rem)
        plan.append(c)
        rem -= c
    while len(plan) >= 3 and plan[-1] == 4:
        plan[-1] = 2
        plan.append(2)

    # per-class EMA scales
    w_all = (1.0 - THETA) / (cls_counts + EPS)  # [C]
    thi_diag = THETA * (cls_counts + EPS) / (1.0 - THETA)  # [C]

    eye8 = np.zeros((P + 1, P), dtype=f8)
    eye8[np.arange(P), np.arange(P)] = 1.0

    in_maps = []
    start = 0
    for i in range(NCORES):
        n_i = int(core_counts[i])
        rows_i = order[start : start + n_i]
        start += n_i
        cls_i = y64[rows_i] - i * CPC  # local class in [0,125)

        # packed [T*128, 1152]: embed fp8 | raw one-hot fp8
        packed = np.zeros((n_pad, W), dtype=f8)
        packed[:n_i, 0:1024] = embed[rows_i].astype(f8)
        packed[:n_i, 1024:1152] = eye8[cls_i]
        # partition-major per tile: [128, T, 1152]
        pm = packed.reshape(T, P, W).transpose(1, 0, 2)

        m = {}
        t0 = 0
        for j, ntj in enumerate(plan):
            m[f"emb{j}"] = np.ascontiguousarray(
                pm[:, t0 : t0 + ntj, :].reshape(P, ntj * W)
            )
            t0 += ntj

        emc = np.zeros((P, W + 128), dtype=np.float16)
        dg = thi_diag[i * CPC : (i + 1) * CPC].astype(np.float16)
        emc[np.arange(CPC), np.arange(CPC)] = dg
        emc[:CPC, 128:1152] = centroid[i * CPC : (i + 1) * CPC].astype(np.float16)
        w32 = np.zeros((P, 1), dtype=np.float32)
        w32[:CPC, 0] = w_all[i * CPC : (i + 1) * CPC].astype(np.float32)
        emc[:, 1152:1154] = w32.view(np.float16)
        m["emc"] = emc
        in_maps.append(m)
    return in_maps, T, cls_counts


def kernel(embed: np.ndarray, y: np.ndarray, centroid: np.ndarray) -> np.ndarray:
    global LAST_RESULTS
    embed = np.ascontiguousarray(np.asarray(embed, dtype=np.float32))
    centroid = np.ascontiguousarray(np.asarray(centroid, dtype=np.float32))

    in_maps, T, cls_counts = _shard_inputs(embed, y, centroid)
    if T not in _NC_CACHE:
        _NC_CACHE[T] = _build_nc(T)
    nc = _NC_CACHE[T]

    trace = os.environ.get("KERNEL_TRACE", "0") == "1"
    res = run_bass_kernel_spmd(
        nc, in_maps, core_ids=list(range(NCORES)), trace=trace
    )
    LAST_RESULTS = res
    out = np.concatenate(
        [res.results[i]["out"][:CPC] for i in range(NCORES)], axis=0
    ).astype(np.float32)
    # empty classes: the fp16 diag underflows; patch exactly on host
    empty = np.where(cls_counts == 0)[0]
    if empty.size:
        out[empty] = THETA * centroid[empty]
    return out
